# revision 10
# baseline (speedup 1.0000x reference)
"""Trainium2 Bass kernel for nn_MultiHeadAttention_59511066853520.

Multi-head attention (H=8 heads, hd=32) with additive relative-position
scores, B=4, S=2048, D=256, fp32.

Math (verified against reference.py):
  scores = (Q K^T) * scale + Q R^T,  R = counts @ rel_table   [S, hd]
  counts[j, b] = #{k : clip(j-k, +-32)+32 == b} is input-independent.
  For j in [32, 2015], counts[j] equals the linear model
    lin[j] = {b=0: 2016-j, b=1..63: 1, b=64: j-31}
  exactly, so the rel part there is a[i] + j*b[i] with
    a = Q @ avec, b = Q @ dvec,
    avec = 2016*T[0] + sum_{1..63} T[b] - 31*T[64],  dvec = T[64]-T[0].
  Softmax shift: c[i] = exact max of the rel part over the 64 candidate
  columns [0..32) u [2016..2048) (the only columns where counts != lin;
  in the middle the rel part is linear so its max is at the boundary
  candidates).  |true rowmax - c| <= |b| + |QK| ~ 10, inside exp's
  fp16/fp32 budget; softmax is shift-invariant.
  bk drops out of softmax exactly; (bv @ Wo + bo) added on host.

Score matmul: a single K=36 fp16 matmul per (head, j-tile, i-chunk):
  stationary rows 0..31 = (scale*K)^T (head dims), rows 32..35 =
  (1, j, 1, j); moving rows 0..31 = Q^T fp16, rows 32..35 =
  (amc_hi, b_hi, amc_lo, b_lo) where amc = a - c split hi/lo in fp16
  (values up to ~4000 need ~fp32 absolute accuracy ~1e-2).
  Heads pair up in the 128x128 PE at row offsets 0 and 64 (64-row
  tiles, K=36 <= 64).  On the two edge j-tiles one extra matmul adds
  sum_b (counts-lin)[j,b] * U[i,b], U = Q @ T^T  (K=65, fp16).

Sharding: core c -> (batch b = c//2, head-group g = c%2 covering heads
4g..4g+3).  Host permutes W* columns to block order [h0, h2, h1, h3]
so heads h0/h1 live at partitions 0..31/64..95 (slots of the paired
matmuls) and h2/h3 reach the second tile pair by sbuf->sbuf DMA.
Each core computes y^T partial; host sums core pairs and transposes.
"""

import sys

if "/opt/trn_rl_repo" not in sys.path:
    sys.path.insert(0, "/opt/trn_rl_repo")

import math
import os

import numpy as np

import concourse.bass as bass
import concourse.bacc as bacc
import concourse.tile as tile
import concourse.mybir as mybir
from concourse import bass_utils

F32 = mybir.dt.float32
F16 = mybir.dt.float16
AF = mybir.ActivationFunctionType
ALU = mybir.AluOpType

B, S, D, H = 4, 2048, 256, 8
HD = D // H            # 32
MAX_REL = 32
VR = 2 * MAX_REL + 1   # 65
SCALE = 1.0 / math.sqrt(HD)
NCORES = 8
NJT = S // 128         # 16 j tiles
NIC = 4                # i chunks
IC = S // NIC          # 512
CAND_JS = list(range(0, 32)) + list(range(2016, 2048))
NCAND = len(CAND_JS)   # 64
EDGE_JT = (0, NJT - 1)
# physical partition block s for quad (pair, half)
SBLK = ((0, 2), (1, 3))


def _counts_matrix():
    # counts[j, b] = #{k in [0,S) : clip(j-k,-32,32)+32 == b}
    j = np.arange(S)
    counts = np.zeros((S, VR), np.float64)
    counts[:, VR - 1] = np.maximum(j - (MAX_REL - 1), 0)   # k <= j-32
    counts[:, 0] = np.maximum(S - MAX_REL - j, 0)          # k >= j+32
    for b in range(1, VR - 1):
        k = j - (b - MAX_REL)
        counts[:, b] = ((k >= 0) & (k < S)).astype(np.float64)
    return counts


def _counts_lin():
    j = np.arange(S, dtype=np.float64)
    lin = np.ones((S, VR), np.float64)
    lin[:, 0] = (S - MAX_REL) - j
    lin[:, VR - 1] = j - (MAX_REL - 1)
    return lin


def build_program():
    dbg = os.environ.get("KDBG", "0") == "1"
    nc = bacc.Bacc("TRN2", target_bir_lowering=False, debug=False)

    def din(name, shape, dt=F32):
        return nc.dram_tensor(name, shape, dt, kind="ExternalInput")

    x_d = din("x", [S, D])
    wq_d = din("wq", [2, 128, 128])
    wk_d = din("wk", [2, 128, 128])          # pre-scaled by SCALE, col-permuted
    wv_d = din("wv", [2, 128, 128])
    bq_d = din("bq", [128])
    woA_d = din("woA", [128, 2, 128], F16)
    woB_d = din("woB", [128, 2, 128], F16)
    cand_d = din("cand", [VR, NCAND], F16)
    corr0_d = din("corr0", [VR, 128], F16)
    corr15_d = din("corr15", [VR, 128], F16)
    relU_d = din("relU", [128, VR], F16)
    abW_d = din("abW", [128, 36])
    auxst_d = din("auxst", [4, S], F16)       # rows (1, j, 1, j)
    mb_d = din("mb", [S])
    identf_d = din("identf", [128, 128])

    ct_dram = nc.dram_tensor("ct_scratch", [64, 128], F32, kind="Internal")
    den_dram = nc.dram_tensor("den_scratch", [2, 2, S], F32, kind="Internal")
    rec_dram = nc.dram_tensor("rec_scratch", [4, S], F32, kind="Internal")
    y_d = nc.dram_tensor("y", [2, 128, S], F32, kind="ExternalOutput")
    if dbg:
        dqa0 = nc.dram_tensor("dqa0", [128, S], F16, kind="ExternalOutput")
        dqa1 = nc.dram_tensor("dqa1", [128, S], F16, kind="ExternalOutput")
        dka0 = nc.dram_tensor("dka0", [128, S], F16, kind="ExternalOutput")
        dka1 = nc.dram_tensor("dka1", [128, S], F16, kind="ExternalOutput")
        du4 = nc.dram_tensor("du4", [VR, 4, S], F16, kind="ExternalOutput")
        dch = nc.dram_tensor("dch", [4, S], F32, kind="ExternalOutput")
        dab = nc.dram_tensor("dab", [36, S], F32, kind="ExternalOutput")
        draw = nc.dram_tensor("draw", [128, S], F32, kind="ExternalOutput")
        dvag = nc.dram_tensor("dvag", [128, 4, NJT, 33], F16, kind="ExternalOutput")

    with tile.TileContext(nc) as tc:
        with (
            tc.tile_pool(name="hold", bufs=1) as hold,
        ):
            # ---- long-lived SBUF tensors ----
            x_nat = hold.tile([128, NJT, D], F32)
            xT = hold.tile([128, 2, S], F32)
            QT = hold.tile([128, S], F32)          # fp32 Q^T (for a/b matmuls)
            QThi = hold.tile([128, S], F16)
            KThi = hold.tile([128, S], F16)        # fp16 (scale*K)^T, block layout
            QTaugP0 = hold.tile([128, S], F16)
            QTaugP1 = hold.tile([128, S], F16)
            KTaugP0 = hold.tile([128, S], F16)
            KTaugP1 = hold.tile([128, S], F16)
            V_aug = hold.tile([128, 4, NJT, 33], F16)
            U4 = hold.tile([VR, 4, S], F16)
            c_all = hold.tile([128, 64], F32)
            ct_sb = hold.tile([64, 128], F32)
            cH = hold.tile([4, S], F32)
            ab_sb = hold.tile([36, S], F32)
            amc = hold.tile([4, S], F32)           # a - c
            amcbh = hold.tile([36, S], F16)        # rows 0..3 amc_hi, 32..35 b_hi
            amcbl = hold.tile([36, S], F16)
            mb_sb = hold.tile([128, NJT], F32)
            rawA = hold.tile([128, S], F32)
            rawB = hold.tile([128, S], F32)
            rec_sm = hold.tile([4, S], F32)
            den_bcA = hold.tile([128, S], F32)
            den_bcB = hold.tile([128, S], F32)
            normA = hold.tile([128, S], F16)
            normB = hold.tile([128, S], F16)
            y_sb = hold.tile([128, 2, S], F32)

            wq_sb = hold.tile([128, 2, 128], F32)
            wk_sb = hold.tile([128, 2, 128], F32)
            wv_sb = hold.tile([128, 2, 128], F32)
            bq_sb = hold.tile([128, 1], F32)
            woA_sb = hold.tile([128, 2, 128], F16)
            woB_sb = hold.tile([128, 2, 128], F16)
            cand_sb = hold.tile([VR, NCAND], F16)
            corr0_sb = hold.tile([VR, 128], F16)
            corr15_sb = hold.tile([VR, 128], F16)
            relU_sb = hold.tile([128, VR], F16)
            abW_sb = hold.tile([128, 36], F32)
            identf_sb = hold.tile([128, 128], F32)

            # ---- input DMAs (x split so transposes start early) ----
            for tq in range(4):
                nc.sync.dma_start(
                    x_nat[:, 4 * tq : 4 * tq + 4, :],
                    x_d.ap().rearrange("(t p) m -> p t m", p=128)[
                        :, 4 * tq : 4 * tq + 4, :
                    ],
                )
            nc.sync.dma_start(wq_sb[:], wq_d.ap().rearrange("k p d -> p k d"))
            nc.sync.dma_start(wk_sb[:], wk_d.ap().rearrange("k p d -> p k d"))
            nc.sync.dma_start(wv_sb[:], wv_d.ap().rearrange("k p d -> p k d"))
            nc.sync.dma_start(bq_sb[:], bq_d.ap().rearrange("(p o) -> p o", o=1))
            nc.sync.dma_start(woA_sb[:], woA_d.ap())
            nc.sync.dma_start(woB_sb[:], woB_d.ap())
            nc.sync.dma_start(cand_sb[:], cand_d.ap())
            nc.sync.dma_start(corr0_sb[:], corr0_d.ap())
            nc.sync.dma_start(corr15_sb[:], corr15_d.ap())
            nc.sync.dma_start(relU_sb[:], relU_d.ap())
            nc.sync.dma_start(abW_sb[:], abW_d.ap())
            nc.sync.dma_start(mb_sb[:], mb_d.ap().rearrange("(t p) -> p t", p=128))
            nc.sync.dma_start(identf_sb[:], identf_d.ap())
            # aux stationary rows (1, j, 1, j) into both KTaug tiles
            for dst in (KTaugP0, KTaugP1):
                nc.sync.dma_start(dst[32:36, :], auxst_d.ap())
                nc.sync.dma_start(dst[96:100, :], auxst_d.ap())

            nc.gpsimd.memset(y_sb[:], 0.0)
            nc.gpsimd.memset(V_aug[:], 1.0)
            nc.gpsimd.memset(rawA[:], 1.0)
            nc.gpsimd.memset(rawB[:], 1.0)
            nc.gpsimd.memset(den_bcA[:], 1.0)
            nc.gpsimd.memset(den_bcB[:], 1.0)

            # ---- phase 1: x^T via PE transposes ----
            with tc.tile_pool(name="pstr", bufs=3, space="PSUM") as pstr:
                for t in range(NJT):
                    xt_ps = pstr.tile([128, 2, 128], F32, tag="xt")
                    for mh in range(2):
                        nc.tensor.transpose(
                            xt_ps[:, mh, :],
                            x_nat[:, t, mh * 128 : (mh + 1) * 128],
                            identf_sb[:],
                        )
                        nc.vector.tensor_copy(
                            xT[:, mh, t * 128 : (t + 1) * 128], xt_ps[:, mh, :]
                        )

            # ---- phase 2a: Q projection (critical path: feeds U/c/a/b) ----
            with tc.tile_pool(name="psp1", bufs=3, space="PSUM") as psp1:
                for ch in range(4):
                    sl = slice(ch * 512, (ch + 1) * 512)
                    q_ps = psp1.tile([128, 512], F32, tag="proj")
                    nc.tensor.matmul(q_ps[:], wq_sb[:, 0, :], xT[:, 0, sl], start=True, stop=False)
                    nc.tensor.matmul(q_ps[:], wq_sb[:, 1, :], xT[:, 1, sl], start=False, stop=True)
                    nc.vector.tensor_scalar_add(QT[:, sl], q_ps[:], bq_sb[:, 0:1])
                    nc.vector.tensor_scalar_add(QThi[:, sl], q_ps[:], bq_sb[:, 0:1])
            # Q data rows of the aug tiles: P0 gets blocks 0/2 in place,
            # P1 gets blocks 1/3 via partition-shifting sbuf->sbuf DMA
            nc.vector.tensor_copy(QTaugP0[0:32, :], QThi[0:32, :])
            nc.vector.tensor_copy(QTaugP0[64:96, :], QThi[64:96, :])
            nc.sync.dma_start(QTaugP1[0:32, :], QThi[32:64, :])
            nc.sync.dma_start(QTaugP1[64:96, :], QThi[96:128, :])

            # ---- phase 3: U4 = (Q @ T^T)^T per block, c candidates, a/b ----
            with tc.tile_pool(name="psu", bufs=4, space="PSUM") as psu:
                for s in range(4):
                    hp = slice(32 * s, 32 * s + 32)
                    for ch in range(4):
                        sl = slice(ch * 512, (ch + 1) * 512)
                        u_ps = psu.tile([VR, 512], F32, tag="u", bufs=2)
                        nc.tensor.matmul(
                            u_ps[:], relU_sb[hp, :], QThi[hp, sl],
                            start=True, stop=True, tile_position=(32 * s, 0),
                        )
                        nc.vector.tensor_copy(U4[:, s, sl], u_ps[:])
                    # candidate max c over the 64 edge columns: all 16 i-tiles
                    # land in one psum tile, one wide max-reduce per block
                    x_ps = psu.tile([128, NJT, NCAND], F32, tag="xc", bufs=1)
                    for it in range(NJT):
                        nc.tensor.matmul(
                            x_ps[:, it, :], U4[:, s, it * 128 : (it + 1) * 128],
                            cand_sb[:], start=True, stop=True,
                        )
                    nc.vector.tensor_reduce(
                        c_all[:, s * NJT : (s + 1) * NJT], x_ps[:],
                        axis=mybir.AxisListType.X, op=ALU.max,
                    )
                # a/b matmuls: rows s = a (block s), rows 32+s = b
                for ch in range(4):
                    sl = slice(ch * 512, (ch + 1) * 512)
                    ab_ps = psu.tile([36, 512], F32, tag="ab", bufs=2)
                    nc.tensor.matmul(
                        ab_ps[:], abW_sb[:], QT[:, sl], start=True, stop=True,
                    )
                    nc.vector.tensor_copy(ab_sb[:, sl], ab_ps[:])
                # transpose c_all -> [64 (s,it), 128 i'], round-trip via DRAM
                # to get cH [4 (s), S]
                ct_ps = psu.tile([64, 128], F32, tag="ct", bufs=1)
                nc.tensor.transpose(ct_ps[:], c_all[:, 0:64], identf_sb[:])
                nc.vector.tensor_copy(ct_sb[:], ct_ps[:])
            nc.sync.dma_start(ct_dram.ap(), ct_sb[:])
            for s in range(4):
                nc.sync.dma_start(
                    cH[s : s + 1, :].rearrange("p (t o) -> p t o", t=NJT),
                    ct_dram.ap()[s * NJT : (s + 1) * NJT, :].rearrange(
                        "(p t) o -> p t o", p=1
                    ),
                )

            def amc_chunk(icc):
                # amc = a - c, hi/lo split, scatter into aux moving rows:
                # block s gets (amc_hi, b_hi, amc_lo, b_lo) at partitions
                # (32 or 96) + 0..3 of its aug tile
                sl = slice(icc * IC, (icc + 1) * IC)
                nc.vector.tensor_tensor(amc[:, sl], ab_sb[0:4, sl], cH[:, sl], op=ALU.subtract)
                nc.vector.tensor_copy(amcbh[0:4, sl], amc[:, sl])
                nc.vector.tensor_copy(amcbh[32:36, sl], ab_sb[32:36, sl])
                nc.vector.tensor_tensor(amcbl[0:4, sl], amc[:, sl], amcbh[0:4, sl], op=ALU.subtract)
                nc.vector.tensor_tensor(amcbl[32:36, sl], ab_sb[32:36, sl], amcbh[32:36, sl], op=ALU.subtract)
                # quad slots: P0 holds blocks (0 @32, 2 @96), P1 (1 @32, 3 @96)
                for s in range(4):
                    dstT = (QTaugP0, QTaugP1, QTaugP0, QTaugP1)[s]
                    poff = (32, 32, 96, 96)[s]
                    nc.sync.dma_start(
                        dstT[poff : poff + 2, sl], amcbh[s : s + 33 : 32, sl]
                    )
                    nc.sync.dma_start(
                        dstT[poff + 2 : poff + 4, sl], amcbl[s : s + 33 : 32, sl]
                    )

            amc_chunk(0)

            # ---- phase 2b: K/V projections + aug assembly ----
            with tc.tile_pool(name="psp2", bufs=3, space="PSUM") as psp2:
                for ch in range(4):
                    sl = slice(ch * 512, (ch + 1) * 512)
                    k_ps = psp2.tile([128, 512], F32, tag="proj")
                    nc.tensor.matmul(k_ps[:], wk_sb[:, 0, :], xT[:, 0, sl], start=True, stop=False)
                    nc.tensor.matmul(k_ps[:], wk_sb[:, 1, :], xT[:, 1, sl], start=False, stop=True)
                    nc.vector.tensor_copy(KThi[:, sl], k_ps[:])
                nc.vector.tensor_copy(KTaugP0[0:32, :], KThi[0:32, :])
                nc.vector.tensor_copy(KTaugP0[64:96, :], KThi[64:96, :])
                nc.sync.dma_start(KTaugP1[0:32, :], KThi[32:64, :])
                nc.sync.dma_start(KTaugP1[64:96, :], KThi[96:128, :])
                for jt in range(NJT):
                    jsl = slice(jt * 128, (jt + 1) * 128)
                    v_ps = psp2.tile([128, 128], F32, tag="vproj")
                    nc.tensor.matmul(v_ps[:], xT[:, 0, jsl], wv_sb[:, 0, :], start=True, stop=False)
                    nc.tensor.matmul(v_ps[:], xT[:, 1, jsl], wv_sb[:, 1, :], start=False, stop=True)
                    nc.vector.tensor_copy(
                        V_aug[:, :, jt, 1:33],
                        v_ps[:].rearrange("p (h d) -> p h d", h=4),
                    )
            for icc in range(1, NIC):
                amc_chunk(icc)

            # ---- phase 4: main attention loop ----
            with (
                tc.tile_pool(name="psq", bufs=2, space="PSUM") as psq,
                tc.tile_pool(name="psacc", bufs=1, space="PSUM") as psacc,
                tc.tile_pool(name="phb", bufs=3) as phb,
            ):
                for ic in range(NIC):
                    isl = slice(ic * IC, (ic + 1) * IC)
                    outA = psacc.tile([128, IC], F32, tag="outA")
                    outB = psacc.tile([128, IC], F32, tag="outB")
                    for jt in range(NJT):
                        jsl = slice(jt * 128, (jt + 1) * 128)
                        edge = jt in EDGE_JT
                        corr_sb = corr0_sb if jt == 0 else corr15_sb
                        for pr in range(2):
                            TK = KTaugP0 if pr == 0 else KTaugP1
                            TQ = QTaugP0 if pr == 0 else QTaugP1
                            quad = psq.tile([128, 2, IC], F32, tag="quad")
                            for half in range(2):
                                poff = 64 * half
                                psl = slice(poff, poff + 36)
                                nc.tensor.matmul(
                                    quad[:, half, :], TK[psl, jsl], TQ[psl, isl],
                                    start=True, stop=not edge,
                                    tile_position=(poff, 0),
                                )
                                if edge:
                                    s = SBLK[pr][half]
                                    nc.tensor.matmul(
                                        quad[:, half, :], corr_sb[:, :],
                                        U4[:, s, isl],
                                        start=False, stop=True,
                                        tile_position=(0, 0),
                                    )
                            ph = phb.tile([128, 2, IC], F16, tag="ph")
                            nc.scalar.activation(
                                ph[:], quad[:], AF.Exp, bias=mb_sb[:, jt : jt + 1]
                            )
                            dst = outA if pr == 0 else outB
                            for half in range(2):
                                s = SBLK[pr][half]
                                cofs = 64 * half
                                nc.tensor.matmul(
                                    dst[cofs : cofs + 33, :],
                                    V_aug[:, s, jt, :], ph[:, half, :],
                                    start=(jt == 0), stop=(jt == NJT - 1),
                                    tile_position=(0, cofs),
                                )
                    # spill unnormalized out + denominators for this i-chunk
                    for srcp, raw, q in ((outA, rawA, 0), (outB, rawB, 1)):
                        nc.vector.tensor_copy(raw[0:33, isl], srcp[0:33, :])
                        nc.vector.tensor_copy(raw[64:97, isl], srcp[64:97, :])
                        # denominators live in raw rows 0 and 64 (ones-row)
                        nc.sync.dma_start(
                            den_dram.ap()[0:1, q, isl], raw[0:1, isl]
                        )
                        nc.sync.dma_start(
                            den_dram.ap()[1:2, q, isl], raw[64:65, isl]
                        )
                    # denominator reciprocal + broadcast + normalize + O-proj
                    nc.sync.dma_start(
                        rec_sm[0:4, isl],
                        den_dram.ap()[:, :, isl].rearrange("a b o -> (a b) o"),
                    )
                    nc.vector.reciprocal(rec_sm[:, isl], rec_sm[:, isl])
                    nc.sync.dma_start(rec_dram.ap()[:, isl], rec_sm[:, isl])
                    for (row, dstt, rows) in (
                        (0, den_bcA, slice(0, 33)),
                        (2, den_bcA, slice(64, 97)),
                        (1, den_bcB, slice(0, 33)),
                        (3, den_bcB, slice(64, 97)),
                    ):
                        nc.sync.dma_start(
                            dstt[rows, isl],
                            rec_dram.ap()[row : row + 1, isl].broadcast_to((33, IC)),
                        )
                    nc.vector.tensor_tensor(normA[:, isl], rawA[:, isl], den_bcA[:, isl], op=ALU.mult)
                    nc.vector.tensor_tensor(normB[:, isl], rawB[:, isl], den_bcB[:, isl], op=ALU.mult)
                    with tc.tile_pool(name="psy", bufs=2, space="PSUM") as psy:
                        for half in range(2):
                            y_ps = psy.tile([128, IC], F32, tag="y")
                            nc.tensor.matmul(y_ps[:], woA_sb[:, half, :], normA[:, isl], start=True, stop=False)
                            nc.tensor.matmul(y_ps[:], woB_sb[:, half, :], normB[:, isl], start=False, stop=True)
                            nc.vector.tensor_copy(y_sb[:, half, isl], y_ps[:])
                    nc.sync.dma_start(
                        y_d.ap().rearrange("k p s -> p k s")[:, :, isl],
                        y_sb[:, :, isl],
                    )
            if dbg:
                nc.sync.dma_start(dqa0.ap(), QTaugP0[:])
                nc.sync.dma_start(dqa1.ap(), QTaugP1[:])
                nc.sync.dma_start(dka0.ap(), KTaugP0[:])
                nc.sync.dma_start(dka1.ap(), KTaugP1[:])
                nc.sync.dma_start(du4.ap(), U4[:])
                nc.sync.dma_start(dch.ap(), cH[:])
                nc.sync.dma_start(dab.ap(), ab_sb[:])
                nc.sync.dma_start(draw.ap(), rawA[:])
                nc.sync.dma_start(dvag.ap(), V_aug[:])

    nc.compile()
    return nc


_CONSTS = None


def _get_consts():
    global _CONSTS
    if _CONSTS is None:
        counts = _counts_matrix()                      # [S, 65] float64
        lin = _counts_lin()
        corr = counts - lin                            # zero for j in [32, 2015]
        jv = np.arange(S, dtype=np.float32)
        auxst = np.stack([np.ones(S, np.float32), jv,
                          np.ones(S, np.float32), jv]).astype(np.float16)
        _CONSTS = {
            "cand": np.ascontiguousarray(
                counts[CAND_JS, :].T.astype(np.float16)),        # [65, 64]
            "corr0": np.ascontiguousarray(
                corr[0:128, :].T.astype(np.float16)),            # [65, 128]
            "corr15": np.ascontiguousarray(
                corr[S - 128 : S, :].T.astype(np.float16)),      # [65, 128]
            "auxst": auxst,                                      # [4, S]
            "identf": np.eye(128, dtype=np.float32),
        }
    return _CONSTS


# physical partition block order: [h0, h2, h1, h3] of the core's head group
_PERM = (
    list(range(0, 32)) + list(range(64, 96))
    + list(range(32, 64)) + list(range(96, 128))
)


def shard_inputs(inputs):
    """Build per-core input maps from full inputs."""
    q = np.asarray(inputs["query"], np.float32)
    mask = np.asarray(inputs["mask"], np.float32)
    Wq = np.asarray(inputs["Wq"], np.float32)
    Wk = np.asarray(inputs["Wk"], np.float32)
    Wv = np.asarray(inputs["Wv"], np.float32)
    Wo = np.asarray(inputs["Wo"], np.float32)
    bq = np.asarray(inputs["bq"], np.float32)
    rel = np.asarray(inputs["rel_table"], np.float32)  # [65, 32]

    c = _get_consts()
    relU = np.tile(rel.T, (4, 1)).astype(np.float16)        # [128, 65]
    avec = (S - MAX_REL) * rel[0] + rel[1 : VR - 1].sum(axis=0) \
        - (MAX_REL - 1) * rel[VR - 1]                        # [32]
    dvec = rel[VR - 1] - rel[0]                              # [32]
    abW = np.zeros((128, 36), np.float32)
    for s in range(4):
        abW[32 * s : 32 * s + 32, s] = avec
        abW[32 * s : 32 * s + 32, 32 + s] = dvec

    in_maps = []
    for core in range(NCORES):
        b, g = core // 2, core % 2
        gc = slice(g * 128, (g + 1) * 128)
        woA = np.zeros((128, 256), np.float32)
        woB = np.zeros((128, 256), np.float32)
        woA[1:33] = Wo[g * 128 + 0 : g * 128 + 32]
        woA[65:97] = Wo[g * 128 + 32 : g * 128 + 64]
        woB[1:33] = Wo[g * 128 + 64 : g * 128 + 96]
        woB[65:97] = Wo[g * 128 + 96 : g * 128 + 128]
        in_maps.append({
            "x": np.ascontiguousarray(q[b]),
            "wq": np.ascontiguousarray(Wq[:, gc][:, _PERM]).reshape(2, 128, 128),
            "wk": np.ascontiguousarray(Wk[:, gc][:, _PERM] * SCALE).reshape(2, 128, 128),
            "wv": np.ascontiguousarray(Wv[:, gc][:, _PERM]).reshape(2, 128, 128),
            "bq": np.ascontiguousarray(bq[gc][_PERM]),
            "woA": woA.reshape(128, 2, 128).astype(np.float16),
            "woB": woB.reshape(128, 2, 128).astype(np.float16),
            "cand": c["cand"],
            "corr0": c["corr0"],
            "corr15": c["corr15"],
            "relU": relU,
            "abW": abW,
            "auxst": c["auxst"],
            "mb": np.ascontiguousarray((1.0 - mask[b, 0, 0, :]) * -1e9 - 4.0),
            "identf": c["identf"],
        })
    return in_maps


def assemble_output(inputs, results):
    """Combine per-core partial y^T into the full [B, S, D] output."""
    Wo = np.asarray(inputs["Wo"], np.float32)
    bo = np.asarray(inputs["bo"], np.float32)
    bv = np.asarray(inputs["bv"], np.float32)
    const_add = bv @ Wo + bo                       # [256]
    y = np.empty((B, S, D), np.float32)
    for b in range(B):
        yt = results[2 * b]["y"] + results[2 * b + 1]["y"]   # [2, 128, S]
        y[b] = yt.reshape(D, S).T + const_add[None, :]
    return y


_PROGRAM = None


def kernel(**inputs) -> np.ndarray:
    global _PROGRAM
    if _PROGRAM is None:
        _PROGRAM = build_program()
    in_maps = shard_inputs(inputs)
    res = bass_utils.run_bass_kernel_spmd(
        _PROGRAM, in_maps, core_ids=list(range(NCORES))
    )
    return assemble_output(inputs, res.results)
